# revision 1
# baseline (speedup 1.0000x reference)
"""GQA sparse attention (packed seqs + sliding window + RoPE) on 8 Trainium2 cores.

Sharding: tensor-parallel over heads. Each of the 8 cores owns 4 Q-heads and
their single shared KV-head (GQA groups stay intact): wq columns
[h*512:(h+1)*512], wk/wv columns [h*128:(h+1)*128], wo rows [h*512:(h+1)*512].
Every core computes a full [S, DIM] partial of the output projection; the host
sums the 8 partials.

The mask never reaches the device: seqlens [1024, 512, 512] with causal +
sliding-window 1024 reduce to block-causal over 128-blocks within each
sequence (the window can never truncate since max causal span == 1024), plus
a causal bias on the diagonal 128x128 blocks.

Per-core dataflow (all matmuls bf16 with fp32 PSUM accumulation):
  qkv:   psum[s,768] = sum_cb xT[cb,si].T @ wqkv[cb]      (weights resident)
  rope:  strided DVE ops on the psum, [s,d] layout, fp32 in / bf16 out
  qT/kT: PE transposes of the roped blocks
  scores(T): psum[sk, sq_span] = kT_blk.T @ qT[h]         (block-causal spans)
  p:     exp(scores + diag_bias) -> pT buffer, bf16       (no max subtraction:
         scores are O(5), exp is safe in fp32)
  pv:    psum[sq, 129] = sum_kj pT_blk.T @ [v_blk | ones] (sums ride along)
  out:   attn = pv[:, :128] * recip(pv[:, 128]),  transpose -> attnT
  wo:    psum[c',s] = sum_db wo[db,cp].T @ attnT[db]      -> DRAM [4096, 2048]
"""

import os

os.environ.setdefault("JAX_PLATFORMS", "axon")

import numpy as np

import concourse.bass as bass
import concourse.mybir as mybir
import concourse.tile as tile
from concourse import bacc
from concourse.bass_utils import run_bass_kernel_spmd

# ---- problem constants (hardcoded per harness contract) ----
DIM = 4096
N_HEADS = 32
N_KV_HEADS = 8
HEAD_DIM = 128
SEQLENS = [1024, 512, 512]
S = 2048
N_CORES = 8
HPC = N_HEADS // N_CORES          # q heads per core = 4
QW = HPC * HEAD_DIM               # per-core q width = 512
KW = HEAD_DIM                     # per-core k/v width = 128
B = 128                           # block size
NSB = S // B                      # 16 seq blocks
NCB = DIM // B                    # 32 contraction blocks
SEQ_BLOCKS = []                   # [(start_blk, end_blk)] per packed sequence
_b = 0
for _l in SEQLENS:
    SEQ_BLOCKS.append((_b, _b + _l // B))
    _b += _l // B

# pT buffer layout: for each kj, columns [offs[kj] : offs[kj]+span(kj)) hold
# p.T for queries sq in [kj*B, seq_end)
_SPANS = {}
_OFFS = {}
_off = 0
for _s0, _s1 in SEQ_BLOCKS:
    for _kj in range(_s0, _s1):
        _SPANS[_kj] = (_s1 - _kj) * B
        _OFFS[_kj] = _off
        _off += _SPANS[_kj]
PT_COLS = _off                    # 7168

F32 = mybir.dt.float32
BF16 = mybir.dt.bfloat16

_PROGRAM = None


def _build_program():
    nc = bacc.Bacc(trn_type="TRN2")

    xt_h = nc.declare_dram_parameter("xt", [NSB, B, DIM], BF16, isOutput=False)
    wqkv_h = nc.declare_dram_parameter("wqkv", [DIM, QW + 2 * KW], BF16, isOutput=False)
    wo_h = nc.declare_dram_parameter("wo", [QW, DIM], BF16, isOutput=False)
    cos_h = nc.declare_dram_parameter("cosr", [NSB, B, 2 * HEAD_DIM], F32, isOutput=False)
    sin_h = nc.declare_dram_parameter("sinr", [NSB, B, 2 * HEAD_DIM], F32, isOutput=False)
    dmask_h = nc.declare_dram_parameter("dmask", [B, B], F32, isOutput=False)
    ident_h = nc.declare_dram_parameter("ident", [B, B], BF16, isOutput=False)
    out_h = nc.declare_dram_parameter("outp", [DIM, S], F32, isOutput=True)

    W768 = QW + 2 * KW  # 768
    Exp = mybir.ActivationFunctionType.Exp

    with tile.TileContext(nc) as tc:
        with (
            tc.tile_pool(name="consts", bufs=1) as cpool,
            tc.tile_pool(name="big", bufs=1) as bigp,
            tc.tile_pool(name="persist", bufs=1) as pers,
            tc.tile_pool(name="ptp", bufs=2) as ptp,
            tc.tile_pool(name="work", bufs=3) as work,
            tc.tile_pool(name="psum", bufs=2, space="PSUM") as psum,
        ):
            # ---- constants ----
            dmask_sb = cpool.tile([B, B], F32)
            nc.sync.dma_start(out=dmask_sb[:], in_=dmask_h[:])
            ident_sb = cpool.tile([B, B], BF16)
            nc.sync.dma_start(out=ident_sb[:], in_=ident_h[:])
            cos_sb = cpool.tile([B, NSB * 2 * HEAD_DIM], F32)
            sin_sb = cpool.tile([B, NSB * 2 * HEAD_DIM], F32)
            for si in range(NSB):
                nc.sync.dma_start(
                    out=cos_sb[:, si * 256:(si + 1) * 256], in_=cos_h[si]
                )
                nc.sync.dma_start(
                    out=sin_sb[:, si * 256:(si + 1) * 256], in_=sin_h[si]
                )

            # ---- resident tensors ----
            wqkv_sb = bigp.tile([B, NCB * W768], BF16, tag="big")
            for cb in range(NCB):
                nc.sync.dma_start(
                    out=wqkv_sb[:, cb * W768:(cb + 1) * W768],
                    in_=wqkv_h[cb * B:(cb + 1) * B, :],
                )
            qT_sb = pers.tile([B, HPC * S], BF16)      # per head h: cols [h*S, (h+1)*S)
            kT_sb = pers.tile([B, S], BF16)
            vaug_sb = pers.tile([B, NSB * 129], BF16)  # per kj: [v_blk | ones]

            # =========== Phase A: qkv projection + rope + transposes ===========
            for si in range(NSB):
                xt_t = work.tile([B, DIM], BF16, tag="xt", bufs=3)
                nc.sync.dma_start(out=xt_t[:], in_=xt_h[si])
                ps = psum.tile([B, W768], F32, tag="A", bufs=2)
                for cb in range(NCB):
                    lhsT = xt_t[:, cb * B:(cb + 1) * B]
                    nc.tensor.matmul(
                        ps[:, 0:512], lhsT, wqkv_sb[:, cb * W768:cb * W768 + 512],
                        start=(cb == 0), stop=(cb == NCB - 1),
                    )
                    nc.tensor.matmul(
                        ps[:, 512:768], lhsT,
                        wqkv_sb[:, cb * W768 + 512:cb * W768 + 768],
                        start=(cb == 0), stop=(cb == NCB - 1),
                    )

                cs = cos_sb[:, si * 256:(si + 1) * 256]
                sn = sin_sb[:, si * 256:(si + 1) * 256]

                # rope on q: [s, d] layout, channels interleaved (even, odd)
                q_t = work.tile([B, QW], BF16, tag="q", bufs=3)
                qe, qo = ps[:, 0:QW:2], ps[:, 1:QW:2]
                t1 = work.tile([B, 256], F32, tag="t1", bufs=2)
                t2 = work.tile([B, 256], F32, tag="t2", bufs=2)
                t3 = work.tile([B, 256], F32, tag="t3", bufs=2)
                t4 = work.tile([B, 256], F32, tag="t4", bufs=2)
                nc.vector.tensor_mul(t1[:], qe, cs)
                nc.vector.tensor_mul(t2[:], qo, sn)
                nc.vector.tensor_sub(q_t[:, 0:QW:2], t1[:], t2[:])
                nc.vector.tensor_mul(t3[:], qe, sn)
                nc.vector.tensor_mul(t4[:], qo, cs)
                nc.vector.tensor_add(q_t[:, 1:QW:2], t3[:], t4[:])

                # rope on k
                k_t = work.tile([B, KW], BF16, tag="k", bufs=3)
                ke, ko = ps[:, 512:640:2], ps[:, 513:640:2]
                c64, s64 = cs[:, 0:64], sn[:, 0:64]
                u1 = work.tile([B, 64], F32, tag="u1", bufs=2)
                u2 = work.tile([B, 64], F32, tag="u2", bufs=2)
                u3 = work.tile([B, 64], F32, tag="u3", bufs=2)
                u4 = work.tile([B, 64], F32, tag="u4", bufs=2)
                nc.vector.tensor_mul(u1[:], ke, c64)
                nc.vector.tensor_mul(u2[:], ko, s64)
                nc.vector.tensor_sub(k_t[:, 0:KW:2], u1[:], u2[:])
                nc.vector.tensor_mul(u3[:], ke, s64)
                nc.vector.tensor_mul(u4[:], ko, c64)
                nc.vector.tensor_add(k_t[:, 1:KW:2], u3[:], u4[:])

                # v block + ones column
                nc.scalar.copy(vaug_sb[:, si * 129:si * 129 + 128], ps[:, 640:768])
                nc.vector.memset(vaug_sb[:, si * 129 + 128:si * 129 + 129], 1.0)

                # transposes: q (4 blocks) and k (1 block)
                for h in range(HPC):
                    tp = psum.tile([B, B], BF16, tag="B", bufs=4)
                    nc.tensor.transpose(tp[:], q_t[:, h * B:(h + 1) * B], ident_sb[:])
                    dst = qT_sb[:, h * S + si * B:h * S + (si + 1) * B]
                    if h % 2 == 0:
                        nc.vector.tensor_copy(dst, tp[:])
                    else:
                        nc.scalar.copy(dst, tp[:])
                ktp = psum.tile([B, B], BF16, tag="B", bufs=4)
                nc.tensor.transpose(ktp[:], k_t[:], ident_sb[:])
                nc.vector.tensor_copy(kT_sb[:, si * B:(si + 1) * B], ktp[:])

            # big2 reuses the wqkv slot: [0:8192)=attnT (by (h, s)), rest = wo
            big2 = bigp.tile([B, NCB * W768], BF16, tag="big")
            attnT_sb = big2[:, 0:HPC * S]
            wo_sb = big2[:, HPC * S:HPC * S + HPC * DIM]
            for db in range(HPC):
                nc.sync.dma_start(
                    out=wo_sb[:, db * DIM:(db + 1) * DIM],
                    in_=wo_h[db * B:(db + 1) * B, :],
                )

            # =========== Phase B: block-sparse attention per head ===========
            for h in range(HPC):
                pT = ptp.tile([B, PT_COLS], BF16, tag="pT", bufs=2)
                for s0, s1 in SEQ_BLOCKS:
                    for kj in range(s0, s1):
                        span = (s1 - kj) * B
                        ps_sc = psum.tile([B, 1024], F32, tag="A", bufs=2)
                        for part in range(0, span, 512):
                            n = min(512, span - part)
                            nc.tensor.matmul(
                                ps_sc[:, part:part + n],
                                kT_sb[:, kj * B:(kj + 1) * B],
                                qT_sb[:, h * S + kj * B + part:
                                      h * S + kj * B + part + n],
                                start=True, stop=True,
                            )
                        # causal bias on the diagonal block
                        nc.vector.tensor_add(
                            ps_sc[:, 0:B], ps_sc[:, 0:B], dmask_sb[:]
                        )
                        nc.scalar.activation(
                            pT[:, _OFFS[kj]:_OFFS[kj] + span], ps_sc[:, 0:span], Exp
                        )
                for s0, s1 in SEQ_BLOCKS:
                    for qi in range(s0, s1):
                        ps_pv = psum.tile([B, 129], F32, tag="B", bufs=4)
                        for kj in range(s0, qi + 1):
                            lhsT = pT[:, _OFFS[kj] + (qi - kj) * B:
                                      _OFFS[kj] + (qi - kj + 1) * B]
                            nc.tensor.matmul(
                                ps_pv[:], lhsT,
                                vaug_sb[:, kj * 129:(kj + 1) * 129],
                                start=(kj == s0), stop=(kj == qi),
                            )
                        rc = work.tile([B, 1], F32, tag="rc", bufs=4)
                        nc.vector.reciprocal(rc[:], ps_pv[:, 128:129])
                        at = work.tile([B, B], BF16, tag="at", bufs=4)
                        nc.scalar.mul(at[:], ps_pv[:, 0:B], rc[:])
                        tp = psum.tile([B, B], BF16, tag="B", bufs=4)
                        nc.tensor.transpose(tp[:], at[:], ident_sb[:])
                        dst = attnT_sb[:, h * S + qi * B:h * S + (qi + 1) * B]
                        if qi % 2 == 0:
                            nc.vector.tensor_copy(dst, tp[:])
                        else:
                            nc.scalar.copy(dst, tp[:])

            # =========== Phase C: output projection (transposed partial) ======
            for cp in range(NCB):
                for scol in range(4):
                    pso = psum.tile([B, 512], F32, tag="B", bufs=4)
                    for db in range(HPC):
                        nc.tensor.matmul(
                            pso[:],
                            wo_sb[:, db * DIM + cp * B:db * DIM + (cp + 1) * B],
                            attnT_sb[:, db * S + scol * 512:db * S + (scol + 1) * 512],
                            start=(db == 0), stop=(db == HPC - 1),
                        )
                    ot = work.tile([B, 512], F32, tag="ot", bufs=4)
                    if scol % 2 == 0:
                        nc.scalar.copy(ot[:], pso[:])
                    else:
                        nc.vector.tensor_copy(ot[:], pso[:])
                    nc.sync.dma_start(
                        out=out_h[cp * B:(cp + 1) * B, scol * 512:(scol + 1) * 512],
                        in_=ot[:],
                    )

    nc.finalize()
    return nc


def get_program():
    global _PROGRAM
    if _PROGRAM is None:
        _PROGRAM = _build_program()
    return _PROGRAM


def make_in_maps(x, cos, sin, wq, wk, wv, wo):
    bf16 = np.dtype("bfloat16") if hasattr(np, "bfloat16") else None
    import ml_dtypes
    bf16 = ml_dtypes.bfloat16

    x = np.asarray(x, np.float32)
    cos = np.asarray(cos, np.float32)
    sin = np.asarray(sin, np.float32)
    wq = np.asarray(wq, np.float32)
    wk = np.asarray(wk, np.float32)
    wv = np.asarray(wv, np.float32)
    wo = np.asarray(wo, np.float32)

    # xt[si, p, cb*B + s] = x[si*B + s, cb*B + p]
    xt = np.ascontiguousarray(
        x.reshape(NSB, B, NCB, B).transpose(0, 3, 2, 1).reshape(NSB, B, DIM)
    ).astype(bf16)
    # cos/sin tiled 4x along channels (per-head repeat), blocked by si
    cosr = np.ascontiguousarray(np.tile(cos, (1, HPC)).reshape(NSB, B, 2 * HEAD_DIM))
    sinr = np.ascontiguousarray(np.tile(sin, (1, HPC)).reshape(NSB, B, 2 * HEAD_DIM))
    # diagonal-block causal bias in scoresT layout: allow sq >= sk
    i = np.arange(B)
    dmask = np.where(i[None, :] >= i[:, None], 0.0, -30000.0).astype(np.float32)
    ident = np.eye(B, dtype=np.float32).astype(bf16)

    scale = HEAD_DIM ** -0.5
    in_maps = []
    for c in range(N_CORES):
        wq_c = (wq[:, c * QW:(c + 1) * QW] * scale).astype(bf16)
        wk_c = wk[:, c * KW:(c + 1) * KW].astype(bf16)
        wv_c = wv[:, c * KW:(c + 1) * KW].astype(bf16)
        wqkv_c = np.ascontiguousarray(
            np.concatenate([wq_c, wk_c, wv_c], axis=1)
        )
        wo_c = np.ascontiguousarray(wo[c * QW:(c + 1) * QW, :]).astype(bf16)
        in_maps.append({
            "xt": xt,
            "wqkv": wqkv_c,
            "wo": wo_c,
            "cosr": cosr,
            "sinr": sinr,
            "dmask": dmask,
            "ident": ident,
        })
    return in_maps


def combine_outputs(results):
    acc = np.zeros((DIM, S), np.float32)
    for r in results:
        acc += np.asarray(r["outp"], np.float32)
    return np.ascontiguousarray(acc.T)


def kernel(x, cos, sin, mask, wq, wk, wv, wo):
    nc = get_program()
    in_maps = make_in_maps(x, cos, sin, wq, wk, wv, wo)
    res = run_bass_kernel_spmd(nc, in_maps, core_ids=list(range(N_CORES)))
    return combine_outputs(res.results)
